# revision 9
# baseline (speedup 1.0000x reference)
"""BinaryXnorExceptOutliersLinear on 8 Trainium2 NeuronCores.

Reference math:
    mask, bscale from global kth-value quantiles of w
    w_q  = per-row asymmetric 8-bit fake quant of w
    w_sim = mask ? w_q : sign(w_q)*bscale
    out  = x @ w_sim.T + bias

Strategy: the weight transform is data-independent of x, so it is done on
the host (numpy, f32, op-for-op like the reference). The device kernel is
a DoubleRow fp8(e4m3) GEMM: per out-row o, codes = w_sim[o,:]/s_o with
s_o = bscale/nb_o and nb_o the largest e4m3-exact value such that
max|codes| <= 240. Non-outliers (+-bscale, 95% of weights) encode EXACTLY
as +-nb_o; only outliers carry e4m3 rounding (~1.3e-2 output rel err).
x is split hi+lo into two fp8 parts packed side by side in the stationary
operand (64 of 128 stationary columns -> DoubleRow max), recovering ~fp16
x precision at zero PE cost. Each core holds its full 8 MiB weight shard
resident in SBUF (64 KiB/partition), streamed via large dependency-free
DMAs on both HW DGE queues; matmuls chase the stream per 256-row k-pair
chunk. Output columns are processed in 2 groups of 512 so group 0's
PSUM drain (DVE copy + DMA out) overlaps group 1's matmuls.
"""
import sys

sys.path.insert(0, "/opt/trn_rl_repo")

import numpy as np
import ml_dtypes
from contextlib import ExitStack

import bass_rust
import concourse.bass as bass
import concourse.mybir as mybir
import concourse.tile as tile
from concourse.bass_utils import run_bass_kernel_spmd

# ---------------------------------------------------------------------------
OUT_F = 8192
IN_F = 8192
BATCH = 32
N_CORES = 8
ROWS_PER_CORE = OUT_F // N_CORES      # 1024
P = 128
CP = IN_F // (2 * P)                   # 32 k-pair chunks of 256
G = 4                                  # output column groups per core
GN = ROWS_PER_CORE // G                # 256 cols per group
OUTLIER_FRACTION = 0.05
F8MAX = 240.0                          # trn float8e4 (IEEE e4m3) max normal

f32 = mybir.dt.float32
f8 = mybir.dt.float8e4
F8NP = ml_dtypes.float8_e4m3

# ---------------------------------------------------------------------------
# walrus compatibility


def _prepare_for_walrus(nc):
    mybir.codegen_inst_isa_subclasses(nc)
    ctr = 0
    for bb in nc.main_func.blocks:
        new = []
        for inst in bb.instructions:
            si = inst.sync_info
            if si is not None and len(si.on_wait) > 1:
                waits = list(si.on_wait)
                for w in waits[:-1]:
                    nop = bass_rust.InstNoOp(
                        name=f"I-wsplit-{ctr}", engine=inst.engine
                    )
                    ctr += 1
                    nop.sync_info = mybir.SyncInfo(on_wait=[w], on_update=[])
                    try:
                        nc.register_instruction(nop, overwrite=True)
                    except Exception:
                        pass
                    new.append(nop)
                si.on_wait = [waits[-1]]
            new.append(inst)
        bb.instructions = new
    return nc


# ---------------------------------------------------------------------------
# device program
#
# psum_g[64, 512] = sum_c  xt[:, c].T @ wt[:, g*32+c]   (DoubleRow, K=256/chunk)
#   stationary xt chunk [128, 2, 64]: cols 0:32 x_hi, 32:64 x_lo
#   moving    wt chunk [128, 2, 512]
# y[64, 1024]: rows 0:32 hi part, 32:64 lo part; host adds halves, applies
# per-col scale s and bias.

# weight DMA pieces in units of 64 KiB (g,c) chunks, alternating SP/ACT in
# consumption order.  Pieces are near-equal so the two queues' descriptor
# streams stay aligned with consumption order (the 16 DMA engines drain
# descriptors in config order — a big late piece configured early would
# delay earlier-needed chunks).  Small head for an early PE start, small
# tail so the final matmuls start right at DMA end.  Byte-balanced with x
# (512 KiB) leading on ACT: SP carries 68 chunks, ACT 60 + x.
SCHEDW = [4, 4, 8, 8, 12, 12, 14, 14, 14, 14, 10, 6, 4, 2, 2]
assert sum(SCHEDW) == G * CP


def _build_nc():
    nc = bass.Bass()
    # host layouts (per-partition contiguous):
    #   wP[p, (g*32+c)*1024 + i*512 + n] = code[g*512+n, c*256+i*128+p]
    #   xS[p, c*128 + i*64 + m]          = xhl[m, c*256+i*128+p]
    wP = nc.dram_tensor("wP", [P, G * CP * 2 * GN], f8, kind="ExternalInput")
    xS = nc.dram_tensor("xS", [P, CP * 2 * 64], f8, kind="ExternalInput")
    y = nc.dram_tensor("y", [64, ROWS_PER_CORE], f32, kind="ExternalOutput")

    PM = mybir.MatmulPerfMode.DoubleRow
    A = mybir.AluOpType

    with tile.TileContext(nc) as tc, ExitStack() as ctx:
        xpool = ctx.enter_context(tc.tile_pool(name="x", bufs=1))
        wpool = ctx.enter_context(tc.tile_pool(name="w", bufs=len(SCHEDW)))
        opool = ctx.enter_context(tc.tile_pool(name="o", bufs=1))
        psum = ctx.enter_context(tc.tile_pool(name="ps", bufs=1, space="PSUM"))

        # x: one piece, first config on the ACT queue
        xt = xpool.tile([P, CP, 2, 64], f8)
        nc.scalar.dma_start(xt[:], xS[:])

        # weight pieces: dependency-free, alternate HW DGE queues
        wts = []           # (gc_start, gc_end, tile)
        c = 0
        for k, wn in enumerate(SCHEDW):
            wt = wpool.tile([P, wn, 2, GN], f8)
            eng = nc.sync if k % 2 == 0 else nc.scalar
            eng.dma_start(wt[:], wP[:, c * 2 * GN:(c + wn) * 2 * GN])
            wts.append((c, c + wn, wt))
            c += wn

        def xchunk(c):
            return xt[:, c]

        def wchunk(gc):
            for a, b, wt in wts:
                if a <= gc < b:
                    return wt[:, gc - a]
            raise AssertionError

        for g in range(G):
            ps = psum.tile([64, GN], f32, tag=f"ps{g}")
            for c in range(CP):
                nc.tensor.matmul(ps[:], xchunk(c), wchunk(g * CP + c),
                                 start=(c == 0), stop=(c == CP - 1),
                                 perf_mode=PM)
            ot = opool.tile([64, GN], f32, tag=f"ot{g}")
            nc.vector.tensor_scalar(ot[:], ps[:], 0.0, None, A.add)
            # outputs go via gpsimd SWDGE so their configs never sit ahead
            # of weight configs in the SP/ACT HW DGE FIFOs (head-of-line
            # blocking).  The last group uses SP: all SP weight configs are
            # long done by then, and HW DGE has a shorter launch path.
            eng = nc.sync if g == G - 1 else nc.gpsimd
            eng.dma_start(y[:, g * GN:(g + 1) * GN], ot[:])

    _prepare_for_walrus(nc)
    return nc


_NC_CACHE = None


def _get_nc():
    global _NC_CACHE
    if _NC_CACHE is None:
        _NC_CACHE = _build_nc()
    return _NC_CACHE


# ---------------------------------------------------------------------------
# host precompute: reference weight transform + e4m3 encoding


def _host_wsim(weight):
    w = np.ascontiguousarray(weight, dtype=np.float32)
    n = w.size
    k_lo = int(n * OUTLIER_FRACTION / 2)
    k_hi = int(n * (1.0 - OUTLIER_FRACTION / 2))
    part = np.partition(w.reshape(-1), [k_lo - 1, k_hi - 1])
    lo = np.float32(part[k_lo - 1])
    hi = np.float32(part[k_hi - 1])
    mask = (w < lo) | (w > hi)
    keep = ~mask
    bscale = np.float32(
        np.sum(np.abs(w) * keep, dtype=np.float32)
        / np.sum(keep, dtype=np.float32)
    )
    # per-row asymmetric 8-bit fake quant, f32 op-for-op like the reference
    w_min = w.min(1, keepdims=True).astype(np.float32)
    w_max = w.max(1, keepdims=True).astype(np.float32)
    rng = (w_max - w_min).astype(np.float32)
    zp = np.round(w_min - np.float32(128.0) * rng / np.float32(255.0)).astype(
        np.float32)
    q = (w - zp).astype(np.float32)
    q = (q * np.float32(255.0)).astype(np.float32)
    q = (q / rng).astype(np.float32)
    q = np.clip(np.round(q), np.float32(0.0), np.float32(255.0)).astype(
        np.float32)
    w_q = (q * (rng / np.float32(255.0)) + zp).astype(np.float32)
    w_sim = np.where(mask, w_q, np.sign(w_q) * bscale).astype(np.float32)
    return w_sim, bscale


def _snap_down_f8(v):
    """Largest e4m3-exact value <= v (v positive normal)."""
    c = v.astype(F8NP)
    cf = c.astype(np.float32)
    bits = c.view(np.uint8)
    bits = np.where(cf > v, bits - 1, bits)
    return bits.view(F8NP).astype(np.float32)


def _encode_f8(w_sim, bscale):
    M = np.abs(w_sim).max(1)
    nb_t = (np.float32(F8MAX) * bscale / M * np.float32(0.999)).astype(
        np.float32)
    nb = _snap_down_f8(nb_t)
    s = (bscale / nb).astype(np.float32)
    codes = np.clip(w_sim / s[:, None], -F8MAX, F8MAX).astype(F8NP)
    return codes, s


def _run(inputs, trace=False):
    x, weight, bias = inputs["x"], inputs["weight"], inputs["bias"]
    w_sim, bscale = _host_wsim(weight)
    codes, s = _encode_f8(w_sim, bscale)

    x2 = np.ascontiguousarray(x, dtype=np.float32).reshape(BATCH, IN_F)
    x_hi = x2.astype(F8NP).astype(np.float32)
    x_lo = (x2 - x_hi).astype(F8NP)
    # xS[p, c*128 + i*64 + m]; m = h*32 + b; k = c*256 + i*128 + p
    st = np.stack([x_hi.astype(F8NP), x_lo], axis=0)   # [h, b, k]
    st = st.reshape(2, BATCH, CP, 2, P)                # [h, b, c, i, p]
    xSv = np.ascontiguousarray(st.transpose(4, 2, 3, 0, 1)).reshape(
        P, CP * 2 * 64)

    nc = _get_nc()
    in_maps = []
    for cid in range(N_CORES):
        sl = slice(cid * ROWS_PER_CORE, (cid + 1) * ROWS_PER_CORE)
        # wP[p, g, c, i, n] = codes_core[g*512+n, c*256+i*128+p]
        cc = codes[sl].reshape(G, GN, CP, 2, P)         # [g, n, c, i, p]
        wPc = np.ascontiguousarray(cc.transpose(4, 0, 2, 3, 1)).reshape(
            P, G * CP * 2 * GN)
        in_maps.append({"wP": wPc, "xS": xSv})
    res = run_bass_kernel_spmd(
        nc, in_maps, core_ids=list(range(N_CORES)), trace=trace
    )
    ys = np.concatenate([r["y"][0:32] + r["y"][32:64] for r in res.results],
                        axis=1)                          # [32, 8192]
    out = (ys * s[None, :] + np.asarray(bias, np.float32)[None, :]).reshape(
        BATCH, 1, OUT_F).astype(np.float32)
    return out, res


def kernel(**inputs):
    out, _ = _run(inputs, trace=False)
    return out
